# revision 9
# baseline (speedup 1.0000x reference)
"""Trainium2 Bass kernel for the Clifford EP model.

The reference model is entirely linear in x_mv:
  * Wx = geometric_product(x, W_in) is linear (Cayley-table contraction).
  * The free-phase relaxation h <- h + dt*(Wx - h), h0 = 0, has the exact
    closed form h_free = (1 - (1-dt)^N) * Wx.
  * The output is the scalar blade of geometric_product(h_free, W_out),
    and C[a, c, 0] != 0 only for c == a.

So the whole network collapses to a single matmul
    out[b, o] = X[b, :] @ Mf[:, o]
with X = x_mv.reshape(B, M*I) and a (M*I, O) folded weight matrix Mf that
only depends on W_in, W_out and the Cayley table.  The fold itself is tiny
(512x4096 @ 4096x64) and is done once on the host in float64; the device
does the batch-sized work: a data-parallel (1024x512)@(512x64) matmul per
NeuronCore, which is purely input-bandwidth bound.

Device layout: each core receives X_shard transposed (k on partitions) so
the TensorEngine can contract over k directly:
    psum[o, b] += Mf_chunk[k,o].T @ XT_chunk[k, b]
accumulated over 4 k-chunks of 128, with the 1024-batch free dim split in
two 512-wide matmuls (one PSUM bank each).

The device data path is fp16 (PSUM accumulation stays fp32): one PE pass
per matmul (fp32 needs LOW/HIGH double passes at half stream rate) and
half the DMA bytes.  Measured end-to-end relative error ~3e-4.
Set dtype="f32" in kernel() for the exact fp32 path.

Raw Bass (no TileContext) with manual semaphores: the Tile scheduler's
drain + double all-engine barrier + semaphore-clear tail costs ~7us,
which is material at this kernel size.
"""

import numpy as np

# Model constants (hardcoded per the problem spec).
B, M_DIM, I_B = 8192, 64, 8
H_DIM, O_DIM = 512, 64
K_DIM = M_DIM * I_B  # 512 contraction size
N_CORES = 8
B_SHARD = B // N_CORES  # 1024
KC = K_DIM // 128  # 4 contraction chunks
BH = B_SHARD // 512  # 2 moving-operand halves
DT, N_FREE = 0.1, 20
G_SIG = [1, 1, 1]

_CACHE = {}


def _cayley():
    n = len(G_SIG)
    I = 2**n
    C = np.zeros((I, I, I), dtype=np.float64)
    for a in range(I):
        for b in range(I):
            s = 0
            for i in range(n):
                if (b >> i) & 1:
                    s += bin(a >> (i + 1)).count("1")
            sign = (-1.0) ** s
            common = a & b
            for i in range(n):
                if (common >> i) & 1:
                    sign *= G_SIG[i]
            C[a, b, a ^ b] = sign
    return C


def _fold_weights(W_in, W_out):
    """Collapse W_in, W_out, Cayley table and the relaxation scale into
    a single (K_DIM, O_DIM) float64 matrix Mf with out = X @ Mf."""
    C = _cayley()
    I = I_B
    s = np.array([C[a, a, 0] for a in range(I)])  # scalar-blade signs
    coef = np.zeros((I, I))
    idx = np.zeros((I, I), dtype=np.int64)
    for a in range(I):
        for k in range(I):
            coef[a, k] = C[a, a ^ k, k]
            idx[a, k] = a ^ k
    W_in64 = np.asarray(W_in, dtype=np.float64)
    W_out64 = np.asarray(W_out, dtype=np.float64)
    # U[h, m, a, k] = C[a, a^k, k] * W_in[h, m, a^k]
    U = coef[None, None, :, :] * W_in64[:, :, idx]
    # W2[h, k, o] = s_k * W_out[o, h, k]
    W2 = s[None, :, None] * np.transpose(W_out64, (1, 2, 0))
    Uf = np.transpose(U, (1, 2, 0, 3)).reshape(M_DIM * I, H_DIM * I)
    c0 = 1.0 - (1.0 - DT) ** N_FREE
    return c0 * (Uf @ W2.reshape(H_DIM * I, O_DIM))


def _install_ntff_hook_shim():
    """This image's `antenv` lacks `axon_hooks`, which bass_utils imports
    when trace=True under axon.  Recreate it, wired to the ctypes NTFF
    profiler that trn_agent_boot ships.  No-op when the real module exists."""
    import sys
    import types

    try:
        import antenv.axon_hooks  # noqa: F401

        return
    except ImportError:
        pass
    try:
        import antenv
        from trn_agent_boot.trn_boot import _ntff_profile_via_ctypes

        hook = _ntff_profile_via_ctypes("/opt/axon/libaxon_pjrt.so")
    except Exception:
        antenv, hook = None, None
    if antenv is None:
        return
    mod = types.ModuleType("antenv.axon_hooks")
    mod.get_axon_ntff_profile_hook = lambda: hook
    mod.set_axon_ntff_profile_hook = lambda h: None
    sys.modules["antenv.axon_hooks"] = mod
    antenv.axon_hooks = mod


def _build_bass(dtype_key, n_warm):
    """Build the single-core SPMD program with raw-bass manual sync."""
    key = ("nc", dtype_key, n_warm)
    if key in _CACHE:
        return _CACHE[key]

    import concourse.bass as bass
    import concourse.mybir as mybir

    f32 = mybir.dt.float32
    dt_in = {"f16": mybir.dt.float16, "f32": f32, "bf16": mybir.dt.bfloat16}[
        dtype_key
    ]

    nc = bass.Bass("TRN2", debug=False)
    # xt is host-packed to [128, KC*B_SHARD]: partition p holds chunk c at
    # free offset c*B_SHARD, so each partition row is 8KB contiguous and the
    # whole input loads with two large full-rate DMAs.
    xt = nc.dram_tensor("xt", [128, KC * B_SHARD], dt_in, kind="ExternalInput")
    # mf is host-prearranged to [128, KC*O_DIM] (contiguous 512B rows) —
    # loading the natural [512, 64] layout needs a 128B-element gather that
    # measured ~3.7us and gated the first matmul.
    mf = nc.dram_tensor("mf", [128, KC * O_DIM], dt_in, kind="ExternalInput")
    # Output is [2*64, 512]: batch-half stacked on partitions, so the copy
    # and store run at full 128-partition width.
    out_t = nc.dram_tensor("out_t", [BH * O_DIM, 512], f32, kind="ExternalOutput")

    with (
        nc.sbuf_tensor([128, KC, B_SHARD], dt_in) as xts,
        nc.sbuf_tensor([128, KC, O_DIM], dt_in) as mft,
        nc.sbuf_tensor([BH * O_DIM, 512], f32) as o_sb,
        nc.psum_tensor([BH * O_DIM, 512], f32) as ps,
        nc.semaphore("sem_mf") as sem_mf,
        nc.semaphore("sem_xt01") as sem_xt01,
        nc.semaphore("sem_xt23") as sem_xt23,
        nc.semaphore("sem_mm") as sem_mm,
        nc.semaphore("sem_cp") as sem_cp,
        nc.semaphore("sem_out") as sem_out,
        nc.Block() as block,
    ):
        half = 2 * B_SHARD  # free-dim elements per DMA half (chunks 0-1 / 2-3)

        @block.sync
        def _(sync):
            sync.dma_start(
                out=xts[:, 0:2, :], in_=xt[:, 0:half]
            ).then_inc(sem_xt01, 16)
            # Store the first half of the result as soon as it is copied.
            sync.wait_ge(sem_cp, 1)
            sync.dma_start(out=out_t[:, 0:256], in_=o_sb[:, 0:256]).then_inc(
                sem_out, 16
            )
            sync.wait_ge(sem_out, 32)

        @block.scalar
        def _(scalar):
            # Second HWDGE issuer, in parallel with sync.
            scalar.dma_start(out=mft[:], in_=mf[:]).then_inc(sem_mf, 16)
            scalar.dma_start(
                out=xts[:, 2:4, :], in_=xt[:, half : 2 * half]
            ).then_inc(sem_xt23, 16)
            scalar.wait_ge(sem_cp, 2)
            scalar.dma_start(out=out_t[:, 256:512], in_=o_sb[:, 256:512]).then_inc(
                sem_out, 16
            )

        @block.tensor
        def _(tensor):
            tensor.wait_ge(sem_mf, 16)
            for kc in range(KC):
                tensor.wait_ge(sem_xt01 if kc < 2 else sem_xt23, 16)
                # The two batch halves go to separate PE column groups
                # (stationary cols 0-63 / 64-127) and run concurrently,
                # accumulating into one [128, 512] PSUM bank.
                for bh in range(BH):
                    mm = nc.tensor.matmul(
                        ps[bh * O_DIM : (bh + 1) * O_DIM, :],
                        mft[:, kc, :],
                        xts[:, kc, bh * 512 : (bh + 1) * 512],
                        start=(kc == 0),
                        stop=(kc == KC - 1),
                        tile_position=(0, bh * O_DIM),
                    )
                    if kc == KC - 1 and bh == BH - 1:
                        mm.then_inc(sem_mm, 1)

        @block.vector
        def _(vector):
            vector.wait_ge(sem_mm, 1)
            nc.vector.tensor_copy(o_sb[:, 0:256], ps[:, 0:256]).then_inc(sem_cp, 1)
            nc.vector.tensor_copy(o_sb[:, 256:512], ps[:, 256:512]).then_inc(
                sem_cp, 1
            )

    _CACHE[key] = nc
    return nc


def kernel(x_mv, W_in, W_out, trace=False, dtype="f16", n_warm=4, **trace_kwargs):
    _install_ntff_hook_shim()
    from concourse.bass_utils import run_bass_kernel_spmd

    np_dt = {"f16": np.float16, "f32": np.float32, "bf16": None}[dtype]
    if np_dt is None:
        import ml_dtypes

        np_dt = ml_dtypes.bfloat16

    x_mv = np.asarray(x_mv, dtype=np.float32)
    Mf = _fold_weights(W_in, W_out)
    # Device layout: mf[p, kc*O+o] = Mf[kc*128+p, o] (contiguous 512B rows).
    mf_dev = np.ascontiguousarray(
        Mf.reshape(KC, 128, O_DIM).transpose(1, 0, 2).reshape(128, KC * O_DIM),
        dtype=np_dt,
    )

    X = x_mv.reshape(B, K_DIM)
    in_maps = []
    for c in range(N_CORES):
        # Device layout: xt[p, c*B_SHARD + b] = X_shard[b, kc*128 + p].
        xs = np.ascontiguousarray(
            X[c * B_SHARD : (c + 1) * B_SHARD]
            .T.astype(np_dt)
            .reshape(KC, 128, B_SHARD)
            .transpose(1, 0, 2)
            .reshape(128, KC * B_SHARD)
        )
        in_maps.append({"xt": xs, "mf": mf_dev})

    nc = _build_bass(dtype, n_warm)
    res = run_bass_kernel_spmd(
        nc, in_maps, core_ids=list(range(N_CORES)), trace=trace, **trace_kwargs
    )
    _CACHE["last_results"] = res

    out = np.empty((B, O_DIM), dtype=np.float32)
    for c in range(N_CORES):
        # out_t is [BH*O, 512]: row bh*O+o, col j  ->  out[c*B_SHARD + bh*512 + j, o]
        ot = res.results[c]["out_t"].reshape(BH, O_DIM, 512)
        for bh in range(BH):
            out[c * B_SHARD + bh * 512 : c * B_SHARD + (bh + 1) * 512] = ot[bh].T
    return out


# revision 11
# speedup vs baseline: 1.0073x; 1.0073x over previous
"""Trainium2 Bass kernel for the Clifford EP model.

The reference model is entirely linear in x_mv:
  * Wx = geometric_product(x, W_in) is linear (Cayley-table contraction).
  * The free-phase relaxation h <- h + dt*(Wx - h), h0 = 0, has the exact
    closed form h_free = (1 - (1-dt)^N) * Wx.
  * The output is the scalar blade of geometric_product(h_free, W_out),
    and C[a, c, 0] != 0 only for c == a.

So the whole network collapses to a single matmul
    out[b, o] = X[b, :] @ Mf[:, o]
with X = x_mv.reshape(B, M*I) and a (M*I, O) folded weight matrix Mf that
only depends on W_in, W_out and the Cayley table.  The fold itself is tiny
(512x4096 @ 4096x64) and is done once on the host in float64; the device
does the batch-sized work: a data-parallel (1024x512)@(512x64) matmul per
NeuronCore, which is purely input-bandwidth bound.

Device layout: each core receives X_shard transposed (k on partitions) so
the TensorEngine can contract over k directly:
    psum[o, b] += Mf_chunk[k,o].T @ XT_chunk[k, b]
accumulated over 4 k-chunks of 128, with the 1024-batch free dim split in
two 512-wide matmuls (one PSUM bank each).

The device data path is fp16 (PSUM accumulation stays fp32): one PE pass
per matmul (fp32 needs LOW/HIGH double passes at half stream rate) and
half the DMA bytes.  Measured end-to-end relative error ~3e-4.
Set dtype="f32" in kernel() for the exact fp32 path.

Raw Bass (no TileContext) with manual semaphores: the Tile scheduler's
drain + double all-engine barrier + semaphore-clear tail costs ~7us,
which is material at this kernel size.
"""

import numpy as np

# Model constants (hardcoded per the problem spec).
B, M_DIM, I_B = 8192, 64, 8
H_DIM, O_DIM = 512, 64
K_DIM = M_DIM * I_B  # 512 contraction size
N_CORES = 8
B_SHARD = B // N_CORES  # 1024
KC = K_DIM // 128  # 4 contraction chunks
BH = B_SHARD // 512  # 2 moving-operand halves
DT, N_FREE = 0.1, 20
G_SIG = [1, 1, 1]

_CACHE = {}


def _cayley():
    n = len(G_SIG)
    I = 2**n
    C = np.zeros((I, I, I), dtype=np.float64)
    for a in range(I):
        for b in range(I):
            s = 0
            for i in range(n):
                if (b >> i) & 1:
                    s += bin(a >> (i + 1)).count("1")
            sign = (-1.0) ** s
            common = a & b
            for i in range(n):
                if (common >> i) & 1:
                    sign *= G_SIG[i]
            C[a, b, a ^ b] = sign
    return C


def _fold_weights(W_in, W_out):
    """Collapse W_in, W_out, Cayley table and the relaxation scale into
    a single (K_DIM, O_DIM) float64 matrix Mf with out = X @ Mf."""
    C = _cayley()
    I = I_B
    s = np.array([C[a, a, 0] for a in range(I)])  # scalar-blade signs
    coef = np.zeros((I, I))
    idx = np.zeros((I, I), dtype=np.int64)
    for a in range(I):
        for k in range(I):
            coef[a, k] = C[a, a ^ k, k]
            idx[a, k] = a ^ k
    W_in64 = np.asarray(W_in, dtype=np.float64)
    W_out64 = np.asarray(W_out, dtype=np.float64)
    # U[h, m, a, k] = C[a, a^k, k] * W_in[h, m, a^k]
    U = coef[None, None, :, :] * W_in64[:, :, idx]
    # W2[h, k, o] = s_k * W_out[o, h, k]
    W2 = s[None, :, None] * np.transpose(W_out64, (1, 2, 0))
    Uf = np.transpose(U, (1, 2, 0, 3)).reshape(M_DIM * I, H_DIM * I)
    c0 = 1.0 - (1.0 - DT) ** N_FREE
    return c0 * (Uf @ W2.reshape(H_DIM * I, O_DIM))


def _install_ntff_hook_shim():
    """This image's `antenv` lacks `axon_hooks`, which bass_utils imports
    when trace=True under axon.  Recreate it, wired to the ctypes NTFF
    profiler that trn_agent_boot ships.  No-op when the real module exists."""
    import sys
    import types

    try:
        import antenv.axon_hooks  # noqa: F401

        return
    except ImportError:
        pass
    try:
        import antenv
        from trn_agent_boot.trn_boot import _ntff_profile_via_ctypes

        hook = _ntff_profile_via_ctypes("/opt/axon/libaxon_pjrt.so")
    except Exception:
        antenv, hook = None, None
    if antenv is None:
        return
    mod = types.ModuleType("antenv.axon_hooks")
    mod.get_axon_ntff_profile_hook = lambda: hook
    mod.set_axon_ntff_profile_hook = lambda h: None
    sys.modules["antenv.axon_hooks"] = mod
    antenv.axon_hooks = mod


def _build_bass(dtype_key, n_warm):
    """Build the single-core SPMD program with raw-bass manual sync."""
    key = ("nc", dtype_key, n_warm)
    if key in _CACHE:
        return _CACHE[key]

    import concourse.bass as bass
    import concourse.mybir as mybir

    f32 = mybir.dt.float32
    dt_in = {"f16": mybir.dt.float16, "f32": f32, "bf16": mybir.dt.bfloat16}[
        dtype_key
    ]

    nc = bass.Bass("TRN2", debug=False)
    # xt is host-packed to [128, KC*B_SHARD]: partition p holds chunk c at
    # free offset c*B_SHARD, so each partition row is 8KB contiguous and the
    # whole input loads with two large full-rate DMAs.
    xt = nc.dram_tensor("xt", [128, KC * B_SHARD], dt_in, kind="ExternalInput")
    # mf is host-prearranged to [128, KC*O_DIM] (contiguous 512B rows) —
    # loading the natural [512, 64] layout needs a 128B-element gather that
    # measured ~3.7us and gated the first matmul.
    mf = nc.dram_tensor("mf", [128, KC * O_DIM], dt_in, kind="ExternalInput")
    # Output is [2*64, 512]: batch-half stacked on partitions, so the copy
    # and store run at full 128-partition width.
    out_t = nc.dram_tensor("out_t", [BH * O_DIM, 512], f32, kind="ExternalOutput")

    with (
        nc.sbuf_tensor([128, KC, B_SHARD], dt_in) as xts,
        nc.sbuf_tensor([128, KC, O_DIM], dt_in) as mft,
        nc.sbuf_tensor([128, 512], mybir.dt.bfloat16) as warm_w,
        nc.sbuf_tensor([BH * O_DIM, 512], f32) as o_sb,
        nc.psum_tensor([BH * O_DIM, 512], f32) as ps,
        nc.psum_tensor([128, 512], f32) as warm_ps,
        nc.semaphore("sem_mf") as sem_mf,
        nc.semaphore("sem_xt01") as sem_xt01,
        nc.semaphore("sem_xt23") as sem_xt23,
        nc.semaphore("sem_mm") as sem_mm,
        nc.semaphore("sem_cp") as sem_cp,
        nc.semaphore("sem_out") as sem_out,
        nc.Block() as block,
    ):
        half = 2 * B_SHARD  # free-dim elements per DMA half (chunks 0-1 / 2-3)

        @block.sync
        def _(sync):
            sync.dma_start(
                out=xts[:, 0:2, :], in_=xt[:, 0:half]
            ).then_inc(sem_xt01, 16)
            # mf is tiny; issued behind xt01 it lands long before the PE
            # needs it, and keeps scalar free to start the other xt half.
            sync.dma_start(out=mft[:], in_=mf[:]).then_inc(sem_mf, 16)
            # Store the first half of the result as soon as it is copied.
            sync.wait_ge(sem_cp, 1)
            sync.dma_start(out=out_t[:, 0:256], in_=o_sb[:, 0:256]).then_inc(
                sem_out, 16
            )
            sync.wait_ge(sem_out, 32)

        @block.scalar
        def _(scalar):
            # Second HWDGE issuer, in parallel with sync.
            scalar.dma_start(
                out=xts[:, 2:4, :], in_=xt[:, half : 2 * half]
            ).then_inc(sem_xt23, 16)
            scalar.wait_ge(sem_cp, 2)
            scalar.dma_start(out=out_t[:, 256:512], in_=o_sb[:, 256:512]).then_inc(
                sem_out, 16
            )

        @block.tensor
        def _(tensor):
            # Warm the PE HAM clock-gate while the DMAs stream (uninitialized
            # SBUF operands — values are irrelevant, the scratch PSUM bank is
            # never read).  The real matmuls then run at 2.4 GHz instead of
            # the cold 1.2 GHz default.
            for _ in range(n_warm):
                nc.tensor.matmul(
                    warm_ps[:], warm_w[:, :128], warm_w[:], start=True, stop=True
                )
            tensor.wait_ge(sem_mf, 16)
            for kc in range(KC):
                tensor.wait_ge(sem_xt01 if kc < 2 else sem_xt23, 16)
                # The two batch halves go to separate PE column groups
                # (stationary cols 0-63 / 64-127) and run concurrently,
                # accumulating into one [128, 512] PSUM bank.
                for bh in range(BH):
                    mm = nc.tensor.matmul(
                        ps[bh * O_DIM : (bh + 1) * O_DIM, :],
                        mft[:, kc, :],
                        xts[:, kc, bh * 512 : (bh + 1) * 512],
                        start=(kc == 0),
                        stop=(kc == KC - 1),
                        tile_position=(0, bh * O_DIM),
                    )
                    if kc == KC - 1 and bh == BH - 1:
                        mm.then_inc(sem_mm, 1)

        @block.vector
        def _(vector):
            vector.wait_ge(sem_mm, 1)
            nc.vector.tensor_copy(o_sb[:, 0:256], ps[:, 0:256]).then_inc(sem_cp, 1)
            nc.vector.tensor_copy(o_sb[:, 256:512], ps[:, 256:512]).then_inc(
                sem_cp, 1
            )

    _CACHE[key] = nc
    return nc


def kernel(x_mv, W_in, W_out, trace=False, dtype="f16", n_warm=8, **trace_kwargs):
    _install_ntff_hook_shim()
    from concourse.bass_utils import run_bass_kernel_spmd

    np_dt = {"f16": np.float16, "f32": np.float32, "bf16": None}[dtype]
    if np_dt is None:
        import ml_dtypes

        np_dt = ml_dtypes.bfloat16

    x_mv = np.asarray(x_mv, dtype=np.float32)
    Mf = _fold_weights(W_in, W_out)
    # Device layout: mf[p, kc*O+o] = Mf[kc*128+p, o] (contiguous 512B rows).
    mf_dev = np.ascontiguousarray(
        Mf.reshape(KC, 128, O_DIM).transpose(1, 0, 2).reshape(128, KC * O_DIM),
        dtype=np_dt,
    )

    X = x_mv.reshape(B, K_DIM)
    in_maps = []
    for c in range(N_CORES):
        # Device layout: xt[p, c*B_SHARD + b] = X_shard[b, kc*128 + p].
        xs = np.ascontiguousarray(
            X[c * B_SHARD : (c + 1) * B_SHARD]
            .T.astype(np_dt)
            .reshape(KC, 128, B_SHARD)
            .transpose(1, 0, 2)
            .reshape(128, KC * B_SHARD)
        )
        in_maps.append({"xt": xs, "mf": mf_dev})

    nc = _build_bass(dtype, n_warm)
    res = run_bass_kernel_spmd(
        nc, in_maps, core_ids=list(range(N_CORES)), trace=trace, **trace_kwargs
    )
    _CACHE["last_results"] = res

    out = np.empty((B, O_DIM), dtype=np.float32)
    for c in range(N_CORES):
        # out_t is [BH*O, 512]: row bh*O+o, col j  ->  out[c*B_SHARD + bh*512 + j, o]
        ot = res.results[c]["out_t"].reshape(BH, O_DIM, 512)
        for bh in range(BH):
            out[c * B_SHARD + bh * 512 : c * B_SHARD + (bh + 1) * 512] = ot[bh].T
    return out
